# revision 22
# baseline (speedup 1.0000x reference)
"""Trainium2 Bass kernel for DifferentiableEdgeWeighting (8-core SPMD).

Sharding: edges partitioned by source node; core c owns sources
[c*6250, (c+1)*6250). Within a core, segments (source nodes) are bin-packed
onto 128 lanes; each lane stores its segments' edges contiguously (segments
padded to EVEN length) so the scatter-softmax becomes segmented scans along
the free dimension and es rows can be fetched one per slot-PAIR.

Per-edge pipeline (4096-slot spans, gathers round-robined over 4 SWDGE
queues so descriptor generation runs on all 8 Q7 cores):
  - es: dma_gather of 512B pair-rows [es|0|es|0] -> cat[:, :, 0:64] halves.
  - ea: dma_gather of 256B pair-packed rows [ea(2k)|ea(2k+1)]; parity select
    written into cat[:, :, 64:128] halves.
  - norm: DVE sub/mul + free-dim reduce (bf16).
  - MLP: one PE transpose per 128-edge tile (cat -> catT), h = cat @ W1'
    with W1' = W1*diag(|W2|) columns sign-sorted; gate logit =
    sum(relu(h)[:, :P]) - sum(relu(h)[:, P:]) via free-dim reduces.
  - softmax: exp -> segmented prefix-scan -> totals broadcast back through a
    reversed scan (free-dim flips via PE identity/anti-identity transposes).
"""

import sys
import numpy as np

sys.path.insert(0, "/opt/trn_rl_repo")

NUM_S = 50000
NUM_A = 50000
D = 64
H = 128
E = 1_000_000
NCORES = 8
S_PER_CORE = NUM_S // NCORES  # 6250
LANES = 128
W = 1024                      # slots per lane (max padded lane load ~1008)
TILE = 128
SPAN = 4096                   # slots per gather span (32 pos columns)
SPC = SPAN // LANES           # 32 pos columns per span
NSPAN = LANES * W // SPAN     # 32
TPB = 8                       # tiles per PSUM batch
NSLOT = LANES * W
NQ = 4                        # SWDGE queues

_compiled = {}


def _lane_pack(counts):
    """Greedy LPT bin-packing of segments onto LANES lanes."""
    import heapq
    lane_of_seg = np.zeros(counts.shape[0], dtype=np.int32)
    order = np.argsort(-counts, kind="stable")
    heap = [(0, l) for l in range(LANES)]
    heapq.heapify(heap)
    for s in order:
        c = int(counts[s])
        if c == 0:
            continue
        load, l = heapq.heappop(heap)
        lane_of_seg[s] = l
        heapq.heappush(heap, (load + c, l))
    return lane_of_seg


def _wrap16(v):
    """dma_gather index layout: index i lives at [i % 16, i // 16],
    replicated across the 8 Q7 cores (partition groups of 16)."""
    w16 = v.reshape(-1, 16).T  # [16, n/16]
    return np.ascontiguousarray(np.tile(w16, (8, 1)).astype(np.int16))


def _prep_core(c, s_idx, a_idx):
    ids = np.nonzero(s_idx // S_PER_CORE == c)[0]
    sl = (s_idx[ids] - c * S_PER_CORE).astype(np.int32)
    av = a_idx[ids].astype(np.int32)

    o = np.argsort(sl, kind="stable")
    ids, sl, av = ids[o], sl[o], av[o]

    counts = np.bincount(sl, minlength=S_PER_CORE)
    pcounts = ((counts + 1) // 2) * 2      # segments padded to even length
    lane_of_seg = _lane_pack(pcounts)

    lane_key = lane_of_seg[sl]
    o2 = np.argsort(lane_key, kind="stable")  # stable keeps seg-contiguity
    ids, sl, av, lane_key = ids[o2], sl[o2], av[o2], lane_key[o2]

    # per-edge slot position within its lane, with even-padded segments
    n = ids.shape[0]
    new_run = np.ones(n, dtype=bool)
    new_run[1:] = (sl[1:] != sl[:-1]) | (lane_key[1:] != lane_key[:-1])
    run_id = np.cumsum(new_run) - 1                      # dense run index
    run_starts = np.nonzero(new_run)[0]
    run_lane = lane_key[run_starts]
    run_count = np.diff(np.append(run_starts, n))
    run_padded = ((run_count + 1) // 2) * 2
    # offset of each run within its lane = cumsum of padded lengths per lane
    run_off = np.zeros(run_starts.shape[0], dtype=np.int64)
    for l in range(LANES):
        m = run_lane == l
        run_off[m] = np.cumsum(run_padded[m]) - run_padded[m]
    lane_load = np.zeros(LANES, dtype=np.int64)
    np.add.at(lane_load, run_lane, run_padded)
    if lane_load.max() > W:
        raise RuntimeError(f"lane overflow: {lane_load.max()} > {W}")

    pos = run_off[run_id] + (np.arange(n) - run_starts[run_id])

    smat = np.full((LANES, W), -1, dtype=np.int32)
    amat = np.zeros((LANES, W), dtype=np.int32)
    orig = np.full((LANES, W), -1, dtype=np.int64)
    smat[lane_key, pos] = sl
    amat[lane_key, pos] = av
    orig[lane_key, pos] = ids
    # pad slot of odd runs gets its segment id (cont-chains through it)
    oddr = run_count & 1 == 1
    smat[run_lane[oddr], (run_off + run_count)[oddr]] = sl[run_starts[oddr]]

    valid = orig >= 0
    same_as_prev = np.zeros((LANES, W), dtype=bool)
    same_as_prev[:, 1:] = smat[:, 1:] == smat[:, :-1]
    cont = same_as_prev.astype(np.float32)
    is_end = np.ones((LANES, W), dtype=bool)
    is_end[:, :-1] = ~same_as_prev[:, 1:]
    endm = is_end.astype(np.float32)
    rev_cont = np.ascontiguousarray((1.0 - endm)[:, ::-1])
    validf = valid.astype(np.float32)
    invalf = (~valid).astype(np.float32)

    # slot stream order: slot t = (lane t%128, pos t//128) -> column-major
    a_slots = np.ascontiguousarray(amat.T).reshape(-1)
    # es pair index per (lane, pairpos): segment of slot (lane, 2k)
    spmat = np.maximum(smat[:, 0::2], 0)
    sp_slots = np.ascontiguousarray(spmat.T).reshape(-1)
    parm = (amat & 1).astype(np.int16)

    return {
        "orig": orig, "valid": valid,
        "cont": cont, "rev_cont": rev_cont, "endm": endm,
        "validf": validf, "invalf": invalf,
        "idx_sp16": _wrap16(sp_slots),
        "idx_a16": _wrap16(a_slots >> 1),
        "parm": parm,
    }


def _split_excess_waits(nc, mybir):
    """Several ISA structs (SWDGE DMA, Ldweights, DVE copies, NoOp) carry a
    single sync-wait slot, but Tile can emit 2+ waits on one instruction.
    Move excess waits onto NoOps inserted just before the instruction on the
    same engine queue (the sequencer stalls on each in order)."""
    fixn = [0]

    def fix_block(blk):
        new_list = []
        for inst in blk.instructions:
            si = inst.sync_info
            if si is not None and si.on_wait and len(si.on_wait) > 1:
                for w in list(si.on_wait[:-1]):
                    nop = mybir.InstNoOp(
                        name=f"I-waitfix-{fixn[0]}",
                        ins=[],
                        outs=[],
                    )
                    fixn[0] += 1
                    nop.engine = inst.engine
                    nop.sync_info = mybir.SyncInfo(on_wait=[w], on_update=[])
                    try:
                        nc.register_instruction(nop, overwrite=True)
                    except Exception:
                        pass
                    new_list.append(nop)
                si.on_wait = [si.on_wait[-1]]
            new_list.append(inst)
        blk.instructions[:] = new_list

    def walk(blocks):
        for b in blocks:
            fix_block(b)
            inner = getattr(b, "blocks", None)
            if inner:
                walk(inner)

    for f in nc.m.functions:
        walk(f.blocks)


def _build_program(P, use_bias):
    from concourse import mybir, library_config
    from concourse.bacc import Bacc
    import concourse.tile as tile
    from concourse import tile_sem_assignment as _tsa
    _tsa.NUM_SWDGE_GLOBAL_SEMS = NQ
    _tsa.NUM_HWDGE_SEMS = 1

    f32 = mybir.dt.float32
    bf16 = mybir.dt.bfloat16
    f16 = mybir.dt.float16
    i16 = mybir.dt.int16
    Alu = mybir.AluOpType
    Act = mybir.ActivationFunctionType

    nc = Bacc(num_swdge_queues=NQ, dynamic_dma_scratch_size=49152)

    NWA = NSLOT // 16
    NWS = NSLOT // 2 // 16
    tab_sp = nc.declare_dram_parameter("tab_sp", [S_PER_CORE, 256], bf16,
                                       isOutput=False)
    tab_a = nc.declare_dram_parameter("tab_a", [NUM_A // 2, 128], bf16,
                                      isOutput=False)
    idx_sp_d = nc.declare_dram_parameter("idx_sp", [LANES, NWS], i16,
                                         isOutput=False)
    idx_a_d = nc.declare_dram_parameter("idx_a", [LANES, NWA], i16,
                                        isOutput=False)
    par_d = nc.declare_dram_parameter("parm", [LANES, W], i16,
                                      isOutput=False)
    cont_d = nc.declare_dram_parameter("cont", [LANES, W], bf16,
                                       isOutput=False)
    rcont_d = nc.declare_dram_parameter("rev_cont", [LANES, W], bf16,
                                        isOutput=False)
    endm_d = nc.declare_dram_parameter("endm", [LANES, W], bf16,
                                       isOutput=False)
    validm_d = nc.declare_dram_parameter("validm", [LANES, W], bf16,
                                         isOutput=False)
    invalm_d = nc.declare_dram_parameter("invalm", [LANES, W], bf16,
                                         isOutput=False)
    w1p_d = nc.declare_dram_parameter("w1p", [H, H], bf16, isOutput=False)
    b1row_d = nc.declare_dram_parameter("b1row", [1, TPB * H], bf16,
                                        isOutput=False)
    scal_d = nc.declare_dram_parameter("scal", [LANES, 2], f32,
                                       isOutput=False)
    identb_d = nc.declare_dram_parameter("identbm", [128, 128], bf16,
                                         isOutput=False)
    ident_d = nc.declare_dram_parameter("identm", [128, 128], f32,
                                        isOutput=False)
    antid_d = nc.declare_dram_parameter("antidm", [128, 128], f32,
                                        isOutput=False)

    wout_d = nc.declare_dram_parameter("wout", [LANES, W], f32, isOutput=True)
    cout_d = nc.declare_dram_parameter("cout", [LANES, W], f32, isOutput=True)

    with tile.TileContext(nc) as tc:
        with (
            nc.allow_low_precision(reason="fp16 reduce outs: 10-bit mantissa"),
            tc.tile_pool(name="persist", bufs=1) as pp,
            tc.tile_pool(name="work", bufs=3) as wp,
            tc.tile_pool(name="spans", bufs=5) as sp_pool,
            tc.tile_pool(name="psA", bufs=2, space="PSUM") as psA,
            tc.tile_pool(name="psB", bufs=2, space="PSUM") as psB,
        ):
            nc.gpsimd.load_library(library_config.mlp)

            idx_sp = pp.tile([LANES, NWS], i16, tag="idx_sp")
            idx_a = pp.tile([LANES, NWA], i16, tag="idx_a")
            parm = pp.tile([LANES, W], i16, tag="parm")
            contm = pp.tile([LANES, W], bf16, tag="contm")
            rcontm = pp.tile([LANES, W], bf16, tag="rcontm")
            endmm = pp.tile([LANES, W], bf16, tag="endmm")
            validm = pp.tile([LANES, W], bf16, tag="validm")
            invalm = pp.tile([LANES, W], bf16, tag="invalm")
            nsq = pp.tile([LANES, W], f16, tag="nsq")
            gposh = pp.tile([LANES, W], f16, tag="gposh")
            gnegh = pp.tile([LANES, W], f16, tag="gnegh")
            nsqf = pp.tile([LANES, W], f32, tag="nsqf")
            costp = pp.tile([LANES, W], f32, tag="costp")
            exv = pp.tile([LANES, W], f32, tag="exv")
            tmp1 = pp.tile([LANES, W], f32, tag="tmp1")
            tmp2 = pp.tile([LANES, W], f32, tag="tmp2")
            w1p = pp.tile([H, H], bf16, tag="w1p")
            scal = pp.tile([LANES, 2], f32, tag="scal")
            ident_b = pp.tile([128, 128], bf16, tag="ident_b")
            ident_f = pp.tile([128, 128], f32, tag="ident_f")
            antid_f = pp.tile([128, 128], f32, tag="antid_f")

            nc.sync.dma_start(out=idx_sp[:, :], in_=idx_sp_d[:, :])
            nc.sync.dma_start(out=idx_a[:, :], in_=idx_a_d[:, :])
            nc.sync.dma_start(out=parm[:, :], in_=par_d[:, :])
            nc.sync.dma_start(out=contm[:, :], in_=cont_d[:, :])
            nc.sync.dma_start(out=rcontm[:, :], in_=rcont_d[:, :])
            nc.sync.dma_start(out=endmm[:, :], in_=endm_d[:, :])
            nc.sync.dma_start(out=validm[:, :], in_=validm_d[:, :])
            nc.sync.dma_start(out=invalm[:, :], in_=invalm_d[:, :])
            nc.sync.dma_start(out=scal[:, :], in_=scal_d[:, :])
            nc.sync.dma_start(out=w1p[:, :], in_=w1p_d[:, :])
            nc.sync.dma_start(out=ident_b[:, :], in_=identb_d[:, :])
            nc.sync.dma_start(out=ident_f[:, :], in_=ident_d[:, :])
            nc.sync.dma_start(out=antid_f[:, :], in_=antid_d[:, :])
            if use_bias:
                b1row = pp.tile([1, TPB * H], bf16, tag="b1row")
                onesc = pp.tile([1, H], bf16, tag="onesc")
                nc.sync.dma_start(out=b1row[:, :], in_=b1row_d[:, :])
                nc.vector.memset(onesc[:, :], 1.0)

            # gather queue assignment: Tile assigns Pool-DMAs to DMASW sem
            # lanes round-robin in program order, so queue must equal
            # ordinal % NQ. Alternate es/ea order per span pair to balance
            # descriptor load (ea has 2x the descriptors of es).
            ordn = [0]

            def gq():
                q = ordn[0] % NQ
                ordn[0] += 1
                return q

            PD = 2                  # spans of gather prefetch
            span_tiles = {}

            def issue_gathers(sp):
                cat = sp_pool.tile([128, SPC // 2, 256], bf16, tag="cat")
                ea_t = sp_pool.tile([128, SPC, 128], bf16, tag="ea_t")
                span_tiles[sp] = (cat, ea_t)
                ia0 = sp * (SPAN // 16)
                g_es = dict(
                    out_ap=cat[:, :, :], in_ap=tab_sp[:, :],
                    idxs_ap=idx_sp[:, sp * (SPAN // 32):(sp + 1) * (SPAN // 32)],
                    num_idxs=SPAN // 2, num_idxs_reg=SPAN // 2, elem_size=256,
                    single_packet=False)
                g_ea_a = dict(
                    out_ap=ea_t[:, 0:SPC // 2, :], in_ap=tab_a[:, :],
                    idxs_ap=idx_a[:, ia0:ia0 + SPAN // 32],
                    num_idxs=SPAN // 2, num_idxs_reg=SPAN // 2, elem_size=128,
                    single_packet=False)
                g_ea_b = dict(
                    out_ap=ea_t[:, SPC // 2:SPC, :], in_ap=tab_a[:, :],
                    idxs_ap=idx_a[:, ia0 + SPAN // 32:ia0 + SPAN // 16],
                    num_idxs=SPAN // 2, num_idxs_reg=SPAN // 2, elem_size=128,
                    single_packet=False)
                nc.gpsimd.dma_gather(queue_num=gq(), **g_ea_a)
                nc.gpsimd.dma_gather(queue_num=gq(), **g_ea_b)
                nc.gpsimd.dma_gather(queue_num=gq(), **g_es)

            def compute_span(sp):
                c0 = sp * SPC          # pos-column base of this span
                cat, ea_t = span_tiles.pop(sp)

                # parity select ea -> cat[:, :, 64:128] / [:, :, 192:256]
                pare = parm[:, c0:c0 + SPC:2].to_broadcast(
                    [128, SPC // 2, 64])
                paro = parm[:, c0 + 1:c0 + SPC:2].to_broadcast(
                    [128, SPC // 2, 64])
                nc.scalar.copy(cat[:, :, 64:128], ea_t[:, 0::2, 0:64])
                nc.vector.copy_predicated(cat[:, :, 64:128], pare,
                                          ea_t[:, 0::2, 64:128])
                nc.scalar.copy(cat[:, :, 192:256], ea_t[:, 1::2, 0:64])
                nc.vector.copy_predicated(cat[:, :, 192:256], paro,
                                          ea_t[:, 1::2, 64:128])

                # norm: diff, square, free-reduce
                dif = wp.tile([128, SPC, 64], bf16, tag="dif")
                nc.vector.tensor_tensor(
                    out=dif[:, 0::2, :], in0=cat[:, :, 0:64],
                    in1=cat[:, :, 64:128], op=Alu.subtract)
                nc.vector.tensor_tensor(
                    out=dif[:, 1::2, :], in0=cat[:, :, 128:192],
                    in1=cat[:, :, 192:256], op=Alu.subtract)
                nc.vector.tensor_tensor(
                    out=dif[:, :, :], in0=dif[:, :, :], in1=dif[:, :, :],
                    op=Alu.mult)
                nc.vector.tensor_reduce(
                    out=nsq[:, c0:c0 + SPC], in_=dif[:, :, :],
                    axis=mybir.AxisListType.X, op=Alu.add)

                for b in range(SPC // TPB):
                    catT_ps = psA.tile([128, TPB, 128], bf16, tag="catT_ps")
                    for i in range(TPB):
                        j = b * TPB + i
                        nc.tensor.transpose(
                            out=catT_ps[:, i, :],
                            in_=cat[:, j // 2, (j % 2) * 128:(j % 2) * 128 + 128],
                            identity=ident_b[:, :])
                    catT = wp.tile([128, TPB, 128], bf16, tag="catT")
                    nc.scalar.copy(catT[:, :, :], catT_ps[:, :, :])

                    h_ps = psB.tile([128, TPB, 128], f32, tag="h_ps")
                    for i in range(TPB):
                        nc.tensor.matmul(
                            h_ps[:, i, :], lhsT=catT[:, i, :], rhs=w1p[:, :],
                            start=(i % 4 == 0), stop=(not use_bias) and (i % 4 == 3),
                            skip_group_check=True)
                    if use_bias:
                        for q in range(TPB // 4):
                            nc.tensor.matmul(
                                h_ps[:, q * 4:(q + 1) * 4, :],
                                lhsT=onesc[:, :],
                                rhs=b1row[:, q * 512:(q + 1) * 512],
                                start=False, stop=True, skip_group_check=True)
                    hr = wp.tile([128, TPB, 128], bf16, tag="hr")
                    nc.scalar.activation(hr[:, :, :], h_ps[:, :, :], Act.Relu)
                    cb = c0 + b * TPB
                    if P > 0:
                        nc.vector.tensor_reduce(
                            out=gposh[:, cb:cb + TPB], in_=hr[:, :, 0:P],
                            axis=mybir.AxisListType.X, op=Alu.add)
                    else:
                        nc.vector.memset(gposh[:, cb:cb + TPB], 0.0)
                    if P < H:
                        nc.vector.tensor_reduce(
                            out=gnegh[:, cb:cb + TPB], in_=hr[:, :, P:H],
                            axis=mybir.AxisListType.X, op=Alu.add)
                    else:
                        nc.vector.memset(gnegh[:, cb:cb + TPB], 0.0)

            for sp in range(NSPAN + PD):
                if sp < NSPAN:
                    issue_gathers(sp)
                if sp >= PD:
                    compute_span(sp - PD)

            # ---------------- packed phase ----------------
            nc.scalar.activation(costp[:, :], nsq[:, :], Act.Sqrt)
            nc.vector.tensor_tensor(out=tmp1[:, :], in0=gposh[:, :],
                                    in1=gnegh[:, :], op=Alu.subtract)
            nc.scalar.activation(tmp2[:, :], tmp1[:, :], Act.Exp,
                                 bias=scal[:, 0:1], scale=-1.0)
            nc.vector.tensor_scalar_add(tmp2[:, :], tmp2[:, :], 1.0)
            nc.vector.reciprocal(tmp1[:, :], tmp2[:, :])            # gate
            nc.vector.tensor_tensor(out=nsqf[:, :], in0=costp[:, :],
                                    in1=tmp1[:, :], op=Alu.mult)    # gated cost
            nc.scalar.activation(exv[:, :], nsqf[:, :], Act.Exp,
                                 bias=0.0, scale=scal[:, 1:2])      # exp(-c/T)
            nc.vector.tensor_tensor(out=exv[:, :], in0=exv[:, :],
                                    in1=validm[:, :], op=Alu.mult)
            nc.vector.tensor_tensor_scan(
                out=tmp1[:, :], data0=contm[:, :], data1=exv[:, :],
                initial=0.0, op0=Alu.mult, op1=Alu.add)             # seg prefix
            nc.vector.tensor_tensor(out=tmp2[:, :], in0=tmp1[:, :],
                                    in1=endmm[:, :], op=Alu.mult)   # ends

            NT = W // 128

            def reverse_free(dst, src):
                for k in range(NT):
                    t_ps = psA.tile([128, 128], f32, tag="catT_ps")
                    nc.tensor.transpose(
                        out=t_ps[:, :],
                        in_=src[:, (NT - 1 - k) * 128:(NT - k) * 128],
                        identity=ident_f[:, :])
                    t_sb = wp.tile([128, 128], f32, tag="t_sb")
                    nc.scalar.copy(t_sb[:, :], t_ps[:, :])
                    t2_ps = psB.tile([128, 128], f32, tag="h_ps")
                    nc.tensor.transpose(out=t2_ps[:, :], in_=t_sb[:, :],
                                        identity=antid_f[:, :])
                    nc.scalar.copy(dst[:, k * 128:(k + 1) * 128], t2_ps[:, :])

            reverse_free(costp, tmp2)                               # drev
            nc.vector.tensor_tensor_scan(
                out=tmp1[:, :], data0=rcontm[:, :], data1=costp[:, :],
                initial=0.0, op0=Alu.mult, op1=Alu.add)             # bcast rev
            reverse_free(tmp2, tmp1)                                # totals/slot
            nc.vector.tensor_tensor(out=tmp2[:, :], in0=tmp2[:, :],
                                    in1=invalm[:, :], op=Alu.add)
            nc.vector.reciprocal(tmp1[:, :], tmp2[:, :])
            nc.vector.tensor_tensor(out=costp[:, :], in0=exv[:, :],
                                    in1=tmp1[:, :], op=Alu.mult)    # weights

            nc.sync.dma_start(out=wout_d[:, :], in_=costp[:, :])
            nc.sync.dma_start(out=cout_d[:, :], in_=nsqf[:, :])

    nc.compile()
    _split_excess_waits(nc, mybir)
    return nc


def _get_program(P, use_bias):
    key = (P, use_bias)
    if key not in _compiled:
        _compiled[key] = _build_program(P, use_bias)
    return _compiled[key]


def _make_in_maps(np_inputs):
    import ml_dtypes
    bf = ml_dtypes.bfloat16
    emb_s = np.asarray(np_inputs["embeddings_s"], dtype=np.float32)
    emb_a = np.asarray(np_inputs["embeddings_a"], dtype=np.float32)
    ei = np.asarray(np_inputs["edge_index_sa"])
    W1 = np.asarray(np_inputs["W1"], dtype=np.float32)
    b1 = np.asarray(np_inputs["b1"], dtype=np.float32).reshape(-1)
    W2v = np.asarray(np_inputs["W2"], dtype=np.float32).reshape(-1)
    b2 = np.asarray(np_inputs["b2"], dtype=np.float32).reshape(-1)
    logt = float(np.asarray(np_inputs["log_temperature"]))

    s_idx = ei[0].astype(np.int64)
    a_idx = ei[1].astype(np.int64)

    temp = float(np.exp(logt))
    w2abs = np.abs(W2v)
    pos = np.nonzero(W2v >= 0)[0]
    neg = np.nonzero(W2v < 0)[0]
    perm = np.concatenate([pos, neg])
    P = int(pos.shape[0])
    W1p = np.ascontiguousarray((W1 * w2abs[None, :])[:, perm]).astype(bf)
    b1p = (b1 * w2abs)[perm].astype(np.float32)
    use_bias = bool(np.any(b1p != 0))
    b1row = np.tile(b1p, TPB).reshape(1, TPB * H).astype(bf)
    scal = np.zeros((LANES, 2), dtype=np.float32)
    scal[:, 0] = -b2[0]
    scal[:, 1] = -1.0 / temp
    identm = np.eye(128, dtype=np.float32)
    identbm = np.eye(128, dtype=np.float32).astype(bf)
    antidm = np.ascontiguousarray(identm[:, ::-1])

    tab_a = np.zeros((NUM_A // 2, 128), dtype=bf)
    ea_b = emb_a.astype(bf)
    tab_a[:, 0:64] = ea_b[0::2]
    tab_a[:, 64:128] = ea_b[1::2]

    in_maps = []
    preps = []
    for c in range(NCORES):
        pr = _prep_core(c, s_idx, a_idx)
        preps.append(pr)
        tab_sp = np.zeros((S_PER_CORE, 256), dtype=bf)
        es_b = emb_s[c * S_PER_CORE:(c + 1) * S_PER_CORE].astype(bf)
        tab_sp[:, 0:64] = es_b
        tab_sp[:, 128:192] = es_b
        in_maps.append({
            "tab_sp": tab_sp,
            "tab_a": tab_a,
            "idx_sp": pr["idx_sp16"],
            "idx_a": pr["idx_a16"],
            "parm": pr["parm"].astype(np.int16),
            "cont": pr["cont"].astype(bf),
            "rev_cont": pr["rev_cont"].astype(bf),
            "endm": pr["endm"].astype(bf),
            "validm": pr["validf"].astype(bf),
            "invalm": pr["invalf"].astype(bf),
            "w1p": W1p,
            "b1row": b1row,
            "scal": scal,
            "identm": identm,
            "identbm": identbm,
            "antidm": antidm,
        })
    return P, use_bias, in_maps, preps


def kernel(embeddings_s, embeddings_a, edge_index_sa, W1, b1, W2, b2,
           log_temperature):
    from concourse.bass_utils import run_bass_kernel_spmd

    np_inputs = {
        "embeddings_s": embeddings_s, "embeddings_a": embeddings_a,
        "edge_index_sa": edge_index_sa, "W1": W1, "b1": b1, "W2": W2,
        "b2": b2, "log_temperature": log_temperature,
    }
    P, use_bias, in_maps, preps = _make_in_maps(np_inputs)
    nc = _get_program(P, use_bias)
    res = run_bass_kernel_spmd(nc, in_maps, core_ids=list(range(NCORES)))

    weights = np.zeros(E, dtype=np.float32)
    costs = np.zeros(E, dtype=np.float32)
    for c in range(NCORES):
        pr = preps[c]
        out = res.results[c]
        v = pr["valid"]
        ids = pr["orig"][v]
        weights[ids] = np.asarray(out["wout"])[v]
        costs[ids] = np.asarray(out["cout"])[v]
    return (weights, costs)
